# revision 1
# baseline (speedup 1.0000x reference)
"""Trainium2 Bass kernel: MoE top-k router (top-8 of 64 experts + softmax).

Contract: kernel(logits, top_k) takes the FULL inputs (logits [1048576, 64]
f32, top_k == 8) and returns (topk_idx int64 [N, 8], topk_w f32 [N, 8]),
matching jax.lax.top_k + jax.nn.softmax semantics (stable descending order,
ties broken toward the smaller index).

Sharding: data-parallel over tokens across 8 NeuronCores (one SPMD program,
per-core slices fed via run_bass_kernel_spmd). Per core, tokens are laid out
partition-major — partition p owns tokens [p*1024, (p+1)*1024) — so every
DMA moves contiguous multi-KB runs per partition.

Per 128-token group the DVE executes one MAX8 (top-8 values, descending,
exact f32 compare) and one MATCH_VALUE_LOAD+FIND_INDEX8 (stable first-match
indices; the HW match unit skips already-matched positions, so duplicate
values get distinct indices in jax order). These three DVE ops are the
bottleneck (~320 ns per 128 tokens); everything else is kept off the DVE:
exp on ScalarE, softmax-denominator tree-sum and the final scale multiply
on GPSIMD. The reciprocal runs on DVE via the 2-instruction ~2ULP
Newton-Raphson approximation (cheaper than the iterative-divide op).
MAX8s are issued in a phase before the FIND pairs so the DVE streams
back-to-back at its ~126 ns/instruction floor. A small first tile (16
tokens/partition) lets the DVE start before the first full 2 MiB tile lands.
"""

import sys

if "/opt/trn_rl_repo" not in sys.path:
    sys.path.insert(0, "/opt/trn_rl_repo")

import numpy as np

N_TOKENS = 1048576
E = 64             # experts
K = 8              # top-k
NCORES = 8
P = 128            # SBUF partitions
TPC = N_TOKENS // NCORES   # tokens per core = 131072
TPP = TPC // P             # tokens per partition = 1024
T = 64                     # tokens per partition per full tile
RAMP = 16                  # first-tile size (earlier DVE start)

_CACHE = {}


def _build(tpp=TPP, t_tile=T, ramp=RAMP):
    import concourse.bacc as bacc
    import concourse.mybir as mybir
    import concourse.tile as tile

    f32 = mybir.dt.float32
    u16 = mybir.dt.uint16

    n_tok = P * tpp
    # graduated small tiles at the start (DVE begins after a 128KB load
    # instead of 2MB) and a small last tile (shorter softmax/store tail
    # after the final DVE instruction)
    if ramp and tpp > 3 * t_tile:
        sizes = ([4, 12, 48] + [t_tile] * (tpp // t_tile - 2)
                 + [t_tile - 16, 16])
    else:
        sizes = [t_tile] * (tpp // t_tile)
    assert sum(sizes) == tpp
    offs = [sum(sizes[:j]) for j in range(len(sizes))]

    nc = bacc.Bacc("TRN2", target_bir_lowering=False, debug=False)
    logits = nc.dram_tensor("logits", [n_tok, E], f32, kind="ExternalInput")
    idx_out = nc.dram_tensor("idx_out", [n_tok, K], u16, kind="ExternalOutput")
    w_out = nc.dram_tensor("w_out", [n_tok, K], f32, kind="ExternalOutput")

    # partition-major: token(p, t) = p*tpp + t
    lg_v = logits.ap().rearrange("(p t) e -> p t e", p=P, t=tpp)
    ix_v = idx_out.ap().rearrange("(p t) k -> p t k", p=P, t=tpp)
    w_v = w_out.ap().rearrange("(p t) k -> p t k", p=P, t=tpp)

    with tile.TileContext(nc) as tc:
        with tc.tile_pool(name="io", bufs=4) as pool:
            for o, tt in zip(offs, sizes):
                x = pool.tile([P, tt, E], f32, tag="x")
                nc.sync.dma_start(x[:], lg_v[:, o:o + tt, :])
                vals = pool.tile([P, tt, K], f32, tag="vals")
                idx = pool.tile([P, tt, K], u16, tag="idx")
                # phase order: all MAX8 first, then the FIND pairs — the DVE
                # then streams each opcode back-to-back without RAW stalls
                for t in range(tt):
                    nc.vector.max(vals[:, t, :], x[:, t, :])
                for t in range(tt):
                    nc.vector.max_index(idx[:, t, :], vals[:, t, :], x[:, t, :])
                ex = pool.tile([P, tt, K], f32, tag="ex")
                nc.scalar.activation(
                    ex[:], vals[:], mybir.ActivationFunctionType.Exp
                )
                # softmax denominator: pairwise tree-sum on GPSIMD (keeps
                # the DVE free for MAX8/FIND_INDEX8, its bottleneck)
                t1 = pool.tile([P, tt, 4], f32, tag="t1")
                t2 = pool.tile([P, tt, 2], f32, tag="t2")
                s = pool.tile([P, tt, 1], f32, tag="s")
                nc.gpsimd.tensor_add(t1[:], ex[:, :, 0:4], ex[:, :, 4:8])
                nc.gpsimd.tensor_add(t2[:], t1[:, :, 0:2], t1[:, :, 2:4])
                nc.gpsimd.tensor_add(s[:], t2[:, :, 0:1], t2[:, :, 1:2])
                # reciprocal: 1-instruction ~51ULP seed on DVE, then one
                # Newton-Raphson refinement (~2ULP) on GPSIMD — only the
                # seed touches the bottleneck engine
                r = pool.tile([P, tt, 1], f32, tag="r")
                if o + tt == tpp:
                    # last tile: its reciprocal chain is serial tail after the
                    # final DVE op — one exact DVE reciprocal (FD=16) beats
                    # seed + 3 GPSIMD refinement ops there
                    nc.vector.reciprocal(r[:], s[:])
                else:
                    rs = pool.tile([P, tt, 1], f32, tag="rs")
                    nc.vector.reciprocal_approx_fast(rs[:], s[:])
                    pq = pool.tile([P, tt, 1], f32, tag="pq")
                    nc.gpsimd.tensor_mul(pq[:], s[:], rs[:])
                    nc.gpsimd.tensor_scalar(pq[:], pq[:], -1.0, 2.0,
                                            op0=mybir.AluOpType.mult,
                                            op1=mybir.AluOpType.add)
                    nc.gpsimd.tensor_mul(r[:], rs[:], pq[:])
                w = pool.tile([P, tt, K], f32, tag="w")
                nc.gpsimd.tensor_mul(w[:], ex[:], r[:].broadcast_to([P, tt, K]))
                nc.sync.dma_start(ix_v[:, o:o + tt, :], idx[:])
                nc.sync.dma_start(w_v[:, o:o + tt, :], w[:])
    nc.compile()
    return nc


def _get_nc():
    if "nc" not in _CACHE:
        _CACHE["nc"] = _build()
    return _CACHE["nc"]


def kernel(logits, top_k):
    logits = np.asarray(logits, dtype=np.float32)
    k = int(np.asarray(top_k))
    assert k == K, f"kernel hardcodes top_k={K}, got {k}"
    assert logits.shape == (N_TOKENS, E), logits.shape

    from concourse.bass_utils import run_bass_kernel_spmd

    nc = _get_nc()
    chunks = logits.reshape(NCORES, TPC, E)
    in_maps = [{"logits": np.ascontiguousarray(chunks[c])} for c in range(NCORES)]
    # The tunneled devices occasionally fail a run with a transient
    # NRT_EXEC_UNIT_UNRECOVERABLE error; a straight retry recovers.
    last_err = None
    for _attempt in range(3):
        try:
            res = run_bass_kernel_spmd(nc, in_maps, list(range(NCORES)))
            break
        except Exception as e:  # noqa: BLE001 - retry transient device faults
            last_err = e
            import time as _time

            _time.sleep(5.0)
    else:
        raise last_err

    # DRAM row r of each per-core output is token r of that core's slice
    # (the views write token p*1024+t at row p*1024+t), so a plain concat
    # along the token axis reassembles the full outputs.
    idx = np.concatenate([r["idx_out"] for r in res.results], axis=0)
    w = np.concatenate([r["w_out"] for r in res.results], axis=0)
    return idx.astype(np.int64), w.astype(np.float32)



# revision 3
# speedup vs baseline: 1.3037x; 1.3037x over previous
"""Trainium2 Bass kernel: MoE top-k router (top-8 of 64 experts + softmax).

Raw-Bass rewrite of the TileContext baseline. Same DVE algorithm (per
128-token column: one MAX8 + one MATCH_VALUE_LOAD/FIND_INDEX8 pair), but
hand-scheduled engine streams with tile-granular semaphores instead of
the framework's per-instruction sync:

- DVE runs a pure max/max_index stream — exactly 2048 instructions per
  core, one satisfied DMA wait and two sem bumps per tile. The softmax
  reciprocal runs entirely on GPSIMD (bitwise-NOT Chebyshev seed + one
  refined + one plain Newton step — the RECIPROCAL_APPROX_FAST math,
  ~51 ULP), so nothing but top-k ever occupies the DVE.
- Act does one Exp per tile (single activation table, loaded once).
- Outputs land in full-size SBUF regions (no ring reuse), so no
  DMA-completion-ordering hazards; inputs use a 4-slot ring with one
  semaphore per slot. The first three ramp tiles are fetched on three
  different engines' DMA queues (SP/Act/Pool) so the ramp loads run in
  parallel and the DVE starts ~2.4us in.

Data layout matches the baseline: per core, tokens are partition-major
(partition p owns tokens [p*1024, (p+1)*1024)), so every DMA moves
contiguous multi-KB runs per partition and a plain concat reassembles
the full output.
"""

import sys

if "/opt/trn_rl_repo" not in sys.path:
    sys.path.insert(0, "/opt/trn_rl_repo")

import numpy as np

N_TOKENS = 1048576
E = 64             # experts
K = 8              # top-k
NCORES = 8
P = 128            # SBUF partitions
TPC = N_TOKENS // NCORES   # tokens per core = 131072
TPP = TPC // P             # tokens per partition = 1024
T = 64                     # tokens per partition per full tile
XRING = 4                  # input ring depth
POOL_OPS = 11              # pool instructions per tile (chain counter)

_CACHE = {}


def _build(tpp=TPP):
    import concourse.bacc as bacc
    import concourse.mybir as mybir

    f32 = mybir.dt.float32
    u16 = mybir.dt.uint16
    u32 = mybir.dt.uint32
    A = mybir.AluOpType
    Act = mybir.ActivationFunctionType

    n_tok = P * tpp
    # graduated ramp (DVE starts after a 64KB load, not 2MB), small tail
    # (short post-DVE softmax/store chain after the last FIND_INDEX8)
    if tpp >= 4 * T:
        sizes = [4, 8, 12, 40] + [T] * (tpp // T - 2) + [32, 16, 8, 8]
    else:
        sizes = [T] * (tpp // T)
    assert sum(sizes) == tpp
    offs = [sum(sizes[:j]) for j in range(len(sizes))]
    N = len(sizes)

    nc = bacc.Bacc("TRN2", target_bir_lowering=False, debug=False)
    logits = nc.dram_tensor("logits", [n_tok, E], f32, kind="ExternalInput")
    idx_out = nc.dram_tensor("idx_out", [n_tok, K], u16, kind="ExternalOutput")
    w_out = nc.dram_tensor("w_out", [n_tok, K], f32, kind="ExternalOutput")

    # partition-major: token(p, t) = p*tpp + t
    lg_v = logits.ap().rearrange("(p t) e -> p t e", p=P, t=tpp)
    ix_v = idx_out.ap().rearrange("(p t) k -> p t k", p=P, t=tpp)
    w_v = w_out.ap().rearrange("(p t) k -> p t k", p=P, t=tpp)

    from contextlib import ExitStack

    with ExitStack() as ctx:
        sb = lambda name, shape, dt: ctx.enter_context(
            nc.sbuf_tensor(name, shape, dt)
        )
        # input ring: 4 slots x 16KB/partition
        x = [sb(f"x{s}", [P, T, E], f32) for s in range(XRING)]
        # full-size (no reuse across tiles)
        vals = sb("vals", [P, tpp, K], f32)
        ex = sb("ex", [P, tpp, K], f32)
        idx = sb("idx", [P, tpp, K], u16)
        w = sb("w", [P, tpp, K], f32)
        # pool-internal scratch (single slots; pool runs in order)
        t1 = sb("t1", [P, T, 4], f32)
        t2 = sb("t2", [P, T, 2], f32)
        s_ = sb("s", [P, T, 1], f32)
        mg = sb("mg", [P, T, 1], u32)
        pp = sb("pp", [P, T, 1], f32)
        rr = sb("rr", [P, T, 1], f32)

        sem = lambda name: ctx.enter_context(nc.semaphore(name))
        dma_in = [sem(f"dma_in{r}") for r in range(XRING)]
        val_rdy = sem("val_rdy")    # DVE: max phase of tile i done  (== i+1)
        dve_fin = sem("dve_fin")    # DVE: find phase of tile i done (== i+1)
        act_sem = sem("act_sem")    # Act: exp of tile i done (== i+1)
        pool_c = sem("pool_c")      # Pool progress: POOL_OPS bumps per tile
        dma_out = sem("dma_out")    # output DMA completions (16 each)

        # fast-reciprocal magic: bits(r0) = MAGIC - bits(s) gives |rel err|
        # <~5%; two Newton steps r <- r*(2 - s*r) finish at ~1e-5.
        MAGIC = 0x7EF0A3D7

        with nc.Block() as block:

            @block.sync
            def _(sync):
                for i in (0, 3):
                    if i < N:
                        sync.dma_start(
                            x[i][:, : sizes[i], :],
                            lg_v[:, offs[i] : offs[i] + sizes[i], :],
                        ).then_inc(dma_in[i], 16)
                for i in range(N):
                    sync.wait_ge(dve_fin, i + 1)
                    j = i + XRING
                    if j < N:
                        sync.dma_start(
                            x[j % XRING][:, : sizes[j], :],
                            lg_v[:, offs[j] : offs[j] + sizes[j], :],
                        ).then_inc(dma_in[j % XRING], 16)
                    o, tt = offs[i], sizes[i]
                    sync.dma_start(
                        ix_v[:, o : o + tt, :], idx[:, o : o + tt, :]
                    ).then_inc(dma_out, 16)
                    if i == N - 1:
                        sync.wait_ge(dve_fin, N + 2)
                    else:
                        sync.wait_ge(pool_c, 1 + POOL_OPS * (i + 1))  # +1: magic memset
                    sync.dma_start(
                        w_v[:, o : o + tt, :], w[:, o : o + tt, :]
                    ).then_inc(dma_out, 16)
                sync.wait_ge(dma_out, 16 * 2 * N)

            @block.vector
            def _(vector):
                for i in range(N):
                    o, tt = offs[i], sizes[i]
                    xs = x[i % XRING]
                    vector.wait_ge(dma_in[i % XRING], 16 * (i // XRING + 1))
                    for t in range(tt):
                        ins = nc.vector.max(vals[:, o + t, :], xs[:, t, :])
                        if t == tt - 1:
                            ins.then_inc(val_rdy, 1)
                    # satisfied instantly (the preceding max bumped it); gives
                    # the race checker the RAW edge for the max->find phases
                    vector.wait_ge(val_rdy, i + 1)
                    for t in range(tt):
                        ins = nc.vector.max_index(
                            idx[:, o + t, :], vals[:, o + t, :], xs[:, t, :]
                        )
                        if t == tt - 1:
                            ins.then_inc(dve_fin, 1)
                    if i == N - 1:
                        # tail: the last tile's reciprocal + w-multiply run on
                        # the DVE - the Pool chain is fixed-overhead bound and
                        # would serialize ~2us after the final FIND_INDEX8
                        vector.wait_ge(pool_c, 1 + POOL_OPS * i + 3)
                        nc.vector.reciprocal(rr[:, :tt, :], s_[:, :tt, :]).then_inc(
                            dve_fin, 1
                        )
                        vector.wait_ge(dve_fin, N + 1)
                        vector.wait_ge(act_sem, i + 1)
                        nc.vector.tensor_mul(
                            w[:, o : o + tt, :],
                            ex[:, o : o + tt, :],
                            rr[:, :tt, :].broadcast_to([P, tt, K]),
                        ).then_inc(dve_fin, 1)

            @block.scalar
            def _(scalar):
                # ramp tiles 1+2 arrive via the Act engine's DMA queue so
                # the first loads overlap (SP's queue serializes its own)
                for i in (1, 2):
                    if i < N:
                        nc.scalar.dma_start(
                            x[i][:, : sizes[i], :],
                            lg_v[:, offs[i] : offs[i] + sizes[i], :],
                        ).then_inc(dma_in[i], 16)
                for i in range(N):
                    o, tt = offs[i], sizes[i]
                    scalar.wait_ge(val_rdy, i + 1)
                    nc.scalar.activation(
                        ex[:, o : o + tt, :], vals[:, o : o + tt, :], Act.Exp
                    ).then_inc(act_sem, 1)

            @block.gpsimd
            def _(gpsimd):
                c = 0

                def chain(ins):
                    nonlocal c
                    c += 1
                    ins.then_inc(pool_c, 1)

                chain(nc.gpsimd.memset(mg[:, :, :], MAGIC))
                for i in range(N):
                    o, tt = offs[i], sizes[i]
                    exi = ex[:, o : o + tt, :]
                    sv = s_[:, :tt, :]
                    ppv, rrv = pp[:, :tt, :], rr[:, :tt, :]
                    gpsimd.wait_ge(act_sem, i + 1)
                    chain(nc.gpsimd.tensor_add(t1[:, :tt, :], exi[:, :, 0:4], exi[:, :, 4:8]))
                    gpsimd.wait_ge(pool_c, c)
                    chain(nc.gpsimd.tensor_add(t2[:, :tt, :], t1[:, :tt, 0:2], t1[:, :tt, 2:4]))
                    gpsimd.wait_ge(pool_c, c)
                    chain(nc.gpsimd.tensor_add(sv, t2[:, :tt, 0:1], t2[:, :tt, 1:2]))
                    if i == N - 1:
                        break  # tail recip+mul run on the DVE (see above)
                    # reciprocal: magic-subtract seed + 2 Newton steps (~1e-5)
                    gpsimd.wait_ge(pool_c, c)
                    chain(nc.gpsimd.tensor_tensor(
                        rrv.bitcast(u32), mg[:, :tt, :], sv.bitcast(u32),
                        op=A.subtract))
                    for _ in range(2):
                        gpsimd.wait_ge(pool_c, c)
                        chain(nc.gpsimd.tensor_mul(ppv, sv, rrv))
                        gpsimd.wait_ge(pool_c, c)
                        chain(nc.gpsimd.tensor_scalar(ppv, ppv, -1.0, 2.0, op0=A.mult, op1=A.add))
                        gpsimd.wait_ge(pool_c, c)
                        chain(nc.gpsimd.tensor_mul(rrv, rrv, ppv))
                    gpsimd.wait_ge(pool_c, c)
                    chain(nc.gpsimd.tensor_mul(
                        w[:, o : o + tt, :], exi, rrv.broadcast_to([P, tt, K])))

    nc.compile()
    return nc


def _get_nc():
    if "nc" not in _CACHE:
        _CACHE["nc"] = _build()
    return _CACHE["nc"]


def kernel(logits, top_k):
    logits = np.asarray(logits, dtype=np.float32)
    k = int(np.asarray(top_k))
    assert k == K, f"kernel hardcodes top_k={K}, got {k}"
    assert logits.shape == (N_TOKENS, E), logits.shape

    from concourse.bass_utils import run_bass_kernel_spmd

    nc = _get_nc()
    chunks = logits.reshape(NCORES, TPC, E)
    in_maps = [{"logits": np.ascontiguousarray(chunks[c])} for c in range(NCORES)]
    # The tunneled devices occasionally fail a run with a transient
    # NRT_EXEC_UNIT_UNRECOVERABLE error; a straight retry recovers.
    last_err = None
    for _attempt in range(3):
        try:
            res = run_bass_kernel_spmd(nc, in_maps, list(range(NCORES)))
            break
        except Exception as e:  # noqa: BLE001 - retry transient device faults
            last_err = e
            import time as _time

            _time.sleep(5.0)
    else:
        raise last_err

    idx = np.concatenate([r["idx_out"] for r in res.results], axis=0)
    w = np.concatenate([r["w_out"] for r in res.results], axis=0)
    return idx.astype(np.int64), w.astype(np.float32)


# revision 5
# speedup vs baseline: 1.3052x; 1.0011x over previous
"""Trainium2 Bass kernel: MoE top-k router (top-8 of 64 experts + softmax).

Raw-Bass rewrite of the TileContext baseline. Same DVE algorithm (per
128-token column: one MAX8 + one MATCH_VALUE_LOAD/FIND_INDEX8 pair), but
hand-scheduled engine streams with tile-granular semaphores instead of
the framework's per-instruction sync:

- DVE runs a pure max/max_index stream — exactly 2048 instructions per
  core, one satisfied DMA wait and two sem bumps per tile. The softmax
  reciprocal runs on GPSIMD (magic-constant seed bits(r0)=MAGIC-bits(s),
  legal on Pool as a u32 subtract, + two Newton steps -> ~1.3e-5 rel
  err), so nothing but top-k occupies the DVE. Only the last tile's
  reciprocal + w-multiply run on the DVE, cutting the post-FIND tail
  from ~2us of fixed-overhead Pool ops to ~250ns.
- Act does one Exp per tile (single activation table, loaded once; Ln
  is avoided deliberately -- Exp+Ln live in different act-table sets and
  alternating them reloads tables at 1.3us a pop).
- Outputs land in full-size SBUF regions (no ring reuse), so no
  DMA-completion-ordering hazards; inputs use a 4-slot ring with one
  semaphore per slot. Ramp tiles 1-2 are fetched on the Act engine's
  DMA queue in parallel with SP's, and the DVE starts ~2.4us in.

Data layout matches the baseline: per core, tokens are partition-major
(partition p owns tokens [p*1024, (p+1)*1024)), so every DMA moves
contiguous multi-KB runs per partition and a plain concat reassembles
the full output.
"""

import sys

if "/opt/trn_rl_repo" not in sys.path:
    sys.path.insert(0, "/opt/trn_rl_repo")

import numpy as np

N_TOKENS = 1048576
E = 64             # experts
K = 8              # top-k
NCORES = 8
P = 128            # SBUF partitions
TPC = N_TOKENS // NCORES   # tokens per core = 131072
TPP = TPC // P             # tokens per partition = 1024
T = 64                     # tokens per partition per full tile
XRING = 4                  # input ring depth
POOL_OPS = 11              # pool instructions per tile (chain counter)

_CACHE = {}


def _build(tpp=TPP):
    import concourse.bacc as bacc
    import concourse.mybir as mybir

    f32 = mybir.dt.float32
    u16 = mybir.dt.uint16
    u32 = mybir.dt.uint32
    A = mybir.AluOpType
    Act = mybir.ActivationFunctionType

    n_tok = P * tpp
    # graduated ramp (DVE starts after a 64KB load, not 2MB), small tail
    # (short post-DVE softmax/store chain after the last FIND_INDEX8)
    if tpp >= 4 * T:
        sizes = [4, 8, 12, 40] + [T] * (tpp // T - 2) + [32, 16, 8, 8]
    else:
        sizes = [T] * (tpp // T)
    assert sum(sizes) == tpp
    offs = [sum(sizes[:j]) for j in range(len(sizes))]
    N = len(sizes)

    nc = bacc.Bacc("TRN2", target_bir_lowering=False, debug=False)
    logits = nc.dram_tensor("logits", [n_tok, E], f32, kind="ExternalInput")
    idx_out = nc.dram_tensor("idx_out", [n_tok, K], u16, kind="ExternalOutput")
    w_out = nc.dram_tensor("w_out", [n_tok, K], f32, kind="ExternalOutput")

    # partition-major: token(p, t) = p*tpp + t
    lg_v = logits.ap().rearrange("(p t) e -> p t e", p=P, t=tpp)
    ix_v = idx_out.ap().rearrange("(p t) k -> p t k", p=P, t=tpp)
    w_v = w_out.ap().rearrange("(p t) k -> p t k", p=P, t=tpp)

    from contextlib import ExitStack

    with ExitStack() as ctx:
        sb = lambda name, shape, dt: ctx.enter_context(
            nc.sbuf_tensor(name, shape, dt)
        )
        # input ring: 4 slots x 16KB/partition
        x = [sb(f"x{s}", [P, T, E], f32) for s in range(XRING)]
        # full-size (no reuse across tiles)
        vals = sb("vals", [P, tpp, K], f32)
        ex = sb("ex", [P, tpp, K], f32)
        idx = sb("idx", [P, tpp, K], u16)
        w = sb("w", [P, tpp, K], f32)
        # pool-internal scratch (single slots; pool runs in order)
        t1 = sb("t1", [P, T, 4], f32)
        t2 = sb("t2", [P, T, 2], f32)
        s_ = sb("s", [P, T, 1], f32)
        mg = sb("mg", [P, T, 1], u32)
        pp = sb("pp", [P, T, 1], f32)
        rr = sb("rr", [P, T, 1], f32)

        sem = lambda name: ctx.enter_context(nc.semaphore(name))
        dma_in = [sem(f"dma_in{r}") for r in range(XRING)]
        val_rdy = sem("val_rdy")    # DVE: max phase of tile i done  (== i+1)
        dve_fin = sem("dve_fin")    # DVE: find phase of tile i done (== i+1)
        act_sem = sem("act_sem")    # Act: exp of tile i done (== i+1)
        pool_c = sem("pool_c")      # Pool progress: POOL_OPS bumps per tile
        dma_out = sem("dma_out")    # output DMA completions (16 each)

        # fast-reciprocal magic: bits(r0) = MAGIC - bits(s) gives |rel err|
        # <~5%; two Newton steps r <- r*(2 - s*r) finish at ~1e-5.
        MAGIC = 0x7EF0A3D7

        with nc.Block() as block:

            @block.sync
            def _(sync):
                for i in (0, 3):
                    if i < N:
                        sync.dma_start(
                            x[i][:, : sizes[i], :],
                            lg_v[:, offs[i] : offs[i] + sizes[i], :],
                        ).then_inc(dma_in[i], 16)
                for i in range(N):
                    sync.wait_ge(dve_fin, i + 1)
                    j = i + XRING
                    if j < N:
                        sync.dma_start(
                            x[j % XRING][:, : sizes[j], :],
                            lg_v[:, offs[j] : offs[j] + sizes[j], :],
                        ).then_inc(dma_in[j % XRING], 16)
                    o, tt = offs[i], sizes[i]
                    sync.dma_start(
                        ix_v[:, o : o + tt, :], idx[:, o : o + tt, :]
                    ).then_inc(dma_out, 16)
                    if i == N - 1:
                        # last tile's w-dma goes out on the idle Act queue
                        # (parallel with this idx dispatch) - see scalar block
                        continue
                    sync.wait_ge(pool_c, 1 + POOL_OPS * (i + 1))  # +1: magic memset
                    sync.dma_start(
                        w_v[:, o : o + tt, :], w[:, o : o + tt, :]
                    ).then_inc(dma_out, 16)
                sync.wait_ge(dma_out, 16 * 2 * N)

            @block.vector
            def _(vector):
                for i in range(N):
                    o, tt = offs[i], sizes[i]
                    xs = x[i % XRING]
                    vector.wait_ge(dma_in[i % XRING], 16 * (i // XRING + 1))
                    for t in range(tt):
                        ins = nc.vector.max(vals[:, o + t, :], xs[:, t, :])
                        if t == tt - 1:
                            ins.then_inc(val_rdy, 1)
                    # satisfied instantly (the preceding max bumped it); gives
                    # the race checker the RAW edge for the max->find phases
                    vector.wait_ge(val_rdy, i + 1)
                    for t in range(tt):
                        ins = nc.vector.max_index(
                            idx[:, o + t, :], vals[:, o + t, :], xs[:, t, :]
                        )
                        if t == tt - 1:
                            ins.then_inc(dve_fin, 1)
                    if i == N - 1:
                        # tail: the last tile's reciprocal + w-multiply run on
                        # the DVE - the Pool chain is fixed-overhead bound and
                        # would serialize ~2us after the final FIND_INDEX8
                        vector.wait_ge(pool_c, 1 + POOL_OPS * i + 3)
                        nc.vector.reciprocal(rr[:, :tt, :], s_[:, :tt, :]).then_inc(
                            dve_fin, 1
                        )
                        vector.wait_ge(dve_fin, N + 1)
                        vector.wait_ge(act_sem, i + 1)
                        nc.vector.tensor_mul(
                            w[:, o : o + tt, :],
                            ex[:, o : o + tt, :],
                            rr[:, :tt, :].broadcast_to([P, tt, K]),
                        ).then_inc(dve_fin, 1)

            @block.scalar
            def _(scalar):
                # ramp tiles 1+2 arrive via the Act engine's DMA queue so
                # the first loads overlap (SP's queue serializes its own)
                for i in (1, 2):
                    if i < N:
                        nc.scalar.dma_start(
                            x[i][:, : sizes[i], :],
                            lg_v[:, offs[i] : offs[i] + sizes[i], :],
                        ).then_inc(dma_in[i], 16)
                for i in range(N):
                    o, tt = offs[i], sizes[i]
                    scalar.wait_ge(val_rdy, i + 1)
                    nc.scalar.activation(
                        ex[:, o : o + tt, :], vals[:, o : o + tt, :], Act.Exp
                    ).then_inc(act_sem, 1)
                o, tt = offs[N - 1], sizes[N - 1]
                scalar.wait_ge(dve_fin, N + 2)
                nc.scalar.dma_start(
                    w_v[:, o : o + tt, :], w[:, o : o + tt, :]
                ).then_inc(dma_out, 16)

            @block.gpsimd
            def _(gpsimd):
                c = 0

                def chain(ins):
                    nonlocal c
                    c += 1
                    ins.then_inc(pool_c, 1)

                chain(nc.gpsimd.memset(mg[:, :, :], MAGIC))
                for i in range(N):
                    o, tt = offs[i], sizes[i]
                    exi = ex[:, o : o + tt, :]
                    sv = s_[:, :tt, :]
                    ppv, rrv = pp[:, :tt, :], rr[:, :tt, :]
                    gpsimd.wait_ge(act_sem, i + 1)
                    chain(nc.gpsimd.tensor_add(t1[:, :tt, :], exi[:, :, 0:4], exi[:, :, 4:8]))
                    gpsimd.wait_ge(pool_c, c)
                    chain(nc.gpsimd.tensor_add(t2[:, :tt, :], t1[:, :tt, 0:2], t1[:, :tt, 2:4]))
                    gpsimd.wait_ge(pool_c, c)
                    chain(nc.gpsimd.tensor_add(sv, t2[:, :tt, 0:1], t2[:, :tt, 1:2]))
                    if i == N - 1:
                        break  # tail recip+mul run on the DVE (see above)
                    # reciprocal: magic-subtract seed + 2 Newton steps (~1e-5)
                    gpsimd.wait_ge(pool_c, c)
                    chain(nc.gpsimd.tensor_tensor(
                        rrv.bitcast(u32), mg[:, :tt, :], sv.bitcast(u32),
                        op=A.subtract))
                    for _ in range(2):
                        gpsimd.wait_ge(pool_c, c)
                        chain(nc.gpsimd.tensor_mul(ppv, sv, rrv))
                        gpsimd.wait_ge(pool_c, c)
                        chain(nc.gpsimd.tensor_scalar(ppv, ppv, -1.0, 2.0, op0=A.mult, op1=A.add))
                        gpsimd.wait_ge(pool_c, c)
                        chain(nc.gpsimd.tensor_mul(rrv, rrv, ppv))
                    gpsimd.wait_ge(pool_c, c)
                    chain(nc.gpsimd.tensor_mul(
                        w[:, o : o + tt, :], exi, rrv.broadcast_to([P, tt, K])))

    nc.compile()
    return nc


def _get_nc():
    if "nc" not in _CACHE:
        _CACHE["nc"] = _build()
    return _CACHE["nc"]


def kernel(logits, top_k):
    logits = np.asarray(logits, dtype=np.float32)
    k = int(np.asarray(top_k))
    assert k == K, f"kernel hardcodes top_k={K}, got {k}"
    assert logits.shape == (N_TOKENS, E), logits.shape

    from concourse.bass_utils import run_bass_kernel_spmd

    nc = _get_nc()
    chunks = logits.reshape(NCORES, TPC, E)
    in_maps = [{"logits": np.ascontiguousarray(chunks[c])} for c in range(NCORES)]
    # The tunneled devices occasionally fail a run with a transient
    # NRT_EXEC_UNIT_UNRECOVERABLE error; a straight retry recovers.
    last_err = None
    for _attempt in range(3):
        try:
            res = run_bass_kernel_spmd(nc, in_maps, list(range(NCORES)))
            break
        except Exception as e:  # noqa: BLE001 - retry transient device faults
            last_err = e
            import time as _time

            _time.sleep(5.0)
    else:
        raise last_err

    idx = np.concatenate([r["idx_out"] for r in res.results], axis=0)
    w = np.concatenate([r["w_out"] for r in res.results], axis=0)
    return idx.astype(np.int64), w.astype(np.float32)
